# revision 35
# baseline (speedup 1.0000x reference)
"""Trainium2 Bass kernel for nn_AutocorrelationCorrelogram (v2).

For nervegram [B=4, F=50, T=20000, C=2]: 300 periodic-Hann-windowed frames
of length 512 per (b,f,c) signal, circular autocorrelation via
Wiener-Khinchin (rfft -> |.|^2 -> irfft), relu, normalize by sqrt(zero
lag), keep 256 lags, mean over channels -> [4, 50, 300, 256].

Sharding: data parallel over the 200 (b,f) pairs -> 25 per core x 8 cores.

v2 structure (per core):
  - HOST pre-frames the signal: xf[c, r, t] bf16 where r = 25*m + bf flat
    frame index (7500 real rows padded to 7680), plus a channel-mean
    un-transpose of the bf16 output; both are cheap numpy ops.
  - DMA-crossbar transposes straight from DRAM: xf[c, rows, :] ->
    yt [128 t_lo, 4 t_chunk, rows] bf16 in SBUF (2 hwdge instrs/chunk,
    no frame-gather DMAs, no PE transposes, no PSUM staging).
  - rfft: bf16 matmuls with the periodic-Hann window folded into the
    DFT matrices; wsin col 0 carries the bin-256 cos column (sin col of
    bin 0 is identically zero).  (An fp8e4 DoubleRow variant exists
    behind rfft_fp8=True; it halves PE streaming but the bf16->fp8
    casts cost as much elsewhere, so it measures the same ~194us at 7x
    worse accuracy - bf16 is the default.)
  - P = Re^2 + Im^2: Squares on ACT, add on DVE -> ph bf16; row-0
    fixups for the bin-256 trick.
  - irfft matmuls use P as the stationary operand so acf lands row-major
    [rows, lags] in PSUM (bf16 weights; fp8 DoubleRow optional but fp8
    on both stages fails the 2e-2 gate).
  - norm: one ACT Abs_reciprocal_sqrt for 1/sqrt(4g*acf0+eps), then
    nt = max(acf*rcc, 0) as DVE scalar_tensor_tensor; channel mean via
    DVE add (0.5 folded into D); one batched bf16 output DMA per chunk.
  - emission is software-pipelined per engine queue: transposes 3 chunks
    ahead (SP), casts 2 ahead, so in-order queues never block front work
    behind tail work; PE keeps its natural rfft -> irfft order.
"""

import sys

import numpy as np

sys.path.insert(0, "/opt/trn_rl_repo")

B, F, T, C = 4, 50, 20000, 2
NUM_FRAME = 300
LEN_FRAME = 512
LAGS = 256
NBINS = 257
N_CORES = 8
BF_PER_CORE = (B * F) // N_CORES  # 25

ROWS = NUM_FRAME * BF_PER_CORE  # 7500 real rows per channel
SB_ROWS = 512
N_SB = 15
RPAD = N_SB * SB_ROWS  # 7680

STARTS = np.linspace(0, T - LEN_FRAME, NUM_FRAME).astype(np.int64)
IDX = STARTS[:, None] + np.arange(LEN_FRAME)  # [300, 512]

WS8 = 32.0  # fp8 rfft weight scale (avoids subnormal W entries)
DS16 = 512.0  # fp16 D scale -> g = 0.25*512/4... A = g*acf with g = 128
DS8 = 131072.0  # fp8 D scale -> g = 0.25*(1/32)*131072 = 1024


def build_weights(rfft_fp8, irfft_fp8):
    import ml_dtypes

    f8 = ml_dtypes.float8_e4m3fn
    t = np.arange(LEN_FRAME, dtype=np.float64)
    win = 0.5 - 0.5 * np.cos(2.0 * np.pi * t / LEN_FRAME)
    ang = 2.0 * np.pi * np.outer(t, np.arange(NBINS)) / LEN_FRAME
    Cm = np.cos(ang) * win[:, None]  # [512, 257]
    Sm = -np.sin(ang) * win[:, None]
    # k-major views [4, 128, 256] with bin-256 trick in wsin col 0
    wc_k = Cm[:, 0:256].reshape(4, 128, 256).copy()
    ws_k = Sm[:, 0:256].reshape(4, 128, 256).copy()
    ws_k[:, :, 0] = Cm[:, 256].reshape(4, 128)

    k = np.arange(NBINS)
    coef = np.full(NBINS, 2.0)
    coef[0] = 1.0
    coef[256] = 1.0
    Dref = (coef[:, None] / LEN_FRAME) * np.cos(
        2.0 * np.pi * np.outer(k, np.arange(LAGS)) / LEN_FRAME
    )  # acf = P @ Dref

    dmdiv = (WS8 * WS8) if rfft_fp8 else 1.0
    w = {}
    if rfft_fp8:
        # [pr, p, pl, m]: t = 128*(2*pr+pl)+p
        w["wc"] = (
            (wc_k * WS8).reshape(2, 2, 128, 256).transpose(0, 2, 1, 3).astype(f8)
        ).copy()
        w["ws"] = (
            (ws_k * WS8).reshape(2, 2, 128, 256).transpose(0, 2, 1, 3).astype(f8)
        ).copy()
    else:
        w["wc"] = wc_k.transpose(1, 0, 2).astype(ml_dtypes.bfloat16).copy()  # [128,4,256]
        w["ws"] = ws_k.transpose(1, 0, 2).astype(ml_dtypes.bfloat16).copy()
    if irfft_fp8:
        dmp = (Dref[:256] * 0.25 * DS8).reshape(2, 128, 256).transpose(1, 0, 2)
        w["dm8"] = dmp.astype(f8).copy()  # [128, 2, 256]
        w["dm2"] = (Dref[256:257] * 0.25 * DS8).astype(ml_dtypes.bfloat16)  # [1, 256]
    else:
        dm = (Dref[:256] * 0.25 * DS16 / dmdiv).astype(ml_dtypes.bfloat16)
        w["dm0"] = dm[0:128].copy()
        w["dm1"] = dm[128:256].copy()
        w["dm2"] = (Dref[256:257] * 0.25 * DS16 / dmdiv).astype(
            ml_dtypes.bfloat16
        )
    return w


def build_nc(rfft_fp8=False, irfft_fp8=False, n_sb=N_SB):
    from contextlib import ExitStack

    import concourse.bacc as bacc
    import concourse.tile as tile
    from concourse import mybir

    f32 = mybir.dt.float32
    f16 = mybir.dt.bfloat16
    f8 = mybir.dt.float8e4
    AF = mybir.ActivationFunctionType
    ALU = mybir.AluOpType
    DRM = mybir.MatmulPerfMode.DoubleRow

    assert not (rfft_fp8 and irfft_fp8), "both-fp8 fails the accuracy gate"

    # scales (see module docstring): A = g*acf, sqrt scale = 4g
    if rfft_fp8:
        # X_psum = WS8*X; squares left unscaled (P_psum = WS8^2 * P) and the
        # 1/WS8^2 is folded into dm0/dm1/dm2 host-side; g is unchanged.
        sact = 1.0
        g = 0.25 * DS16
    elif irfft_fp8:
        sact = float(np.sqrt(1.0 / 32.0))  # sq = X^2/32 (fp8 range)
        g = 0.25 * (1.0 / 32.0) * DS8
    else:
        sact = 1.0
        g = 0.25 * DS16
    s4g = 4.0 * g

    nc = bacc.Bacc("TRN2", target_bir_lowering=False, debug=False)

    xf = nc.dram_tensor("xf", [C, RPAD, LEN_FRAME], f16, kind="ExternalInput").ap()
    if rfft_fp8:
        wc_d = nc.dram_tensor("wc", [2, 128, 2, 256], f8, kind="ExternalInput").ap()
        ws_d = nc.dram_tensor("ws", [2, 128, 2, 256], f8, kind="ExternalInput").ap()
    else:
        wc_d = nc.dram_tensor("wc", [128, 4, 256], f16, kind="ExternalInput").ap()
        ws_d = nc.dram_tensor("ws", [128, 4, 256], f16, kind="ExternalInput").ap()
    if irfft_fp8:
        dm8_d = nc.dram_tensor("dm8", [128, 2, 256], f8, kind="ExternalInput").ap()
    else:
        dm0_d = nc.dram_tensor("dm0", [128, 256], f16, kind="ExternalInput").ap()
        dm1_d = nc.dram_tensor("dm1", [128, 256], f16, kind="ExternalInput").ap()
    dm2_d = nc.dram_tensor("dm2", [1, 256], f16, kind="ExternalInput").ap()
    out = nc.dram_tensor("out", [RPAD, LAGS], f16, kind="ExternalOutput").ap()

    phdt = f8 if irfft_fp8 else f16

    with tile.TileContext(nc) as tc, ExitStack() as ctx:
        consts = ctx.enter_context(tc.tile_pool(name="consts", bufs=1))
        work = ctx.enter_context(tc.tile_pool(name="work", bufs=1))
        pp = ctx.enter_context(tc.tile_pool(name="ps", bufs=1, space="PSUM"))

        # ---- constants (loads issued after the first transposes) ----
        def load_consts():
            if rfft_fp8:
                for pr in range(2):
                    nc.sync.dma_start(out=wc_sb[:, pr], in_=wc_d[pr])
                    nc.sync.dma_start(out=ws_sb[:, pr], in_=ws_d[pr])
            else:
                nc.sync.dma_start(out=wc_sb[:], in_=wc_d[:])
                nc.sync.dma_start(out=ws_sb[:], in_=ws_d[:])
            if irfft_fp8:
                nc.sync.dma_start(out=dm8[:], in_=dm8_d[:])
            else:
                nc.sync.dma_start(out=dm0[:], in_=dm0_d[:])
                nc.sync.dma_start(out=dm1[:], in_=dm1_d[:])
            nc.sync.dma_start(out=dm2[:], in_=dm2_d[:])

        if rfft_fp8:
            wc_sb = consts.tile([128, 2, 2, 256], f8, tag="wc")
            ws_sb = consts.tile([128, 2, 2, 256], f8, tag="ws")
        else:
            wc_sb = consts.tile([128, 4, 256], f16, tag="wc")
            ws_sb = consts.tile([128, 4, 256], f16, tag="ws")
        if irfft_fp8:
            dm8 = consts.tile([128, 2, 256], f8, tag="dm8")
        else:
            dm0 = consts.tile([128, 256], f16, tag="dm0")
            dm1 = consts.tile([128, 256], f16, tag="dm1")
        dm2 = consts.tile([1, 256], f16, tag="dm2")
        zero_b = consts.tile([128, 1], f32, tag="zerob")
        nc.vector.memset(zero_b[:], 0.0)
        eps_b = consts.tile([128, 1], f32, tag="epsb")
        nc.vector.memset(eps_b[:], 1e-30)
        zeros_t = consts.tile([128, SB_ROWS], f16, tag="zerost")
        nc.vector.memset(zeros_t[:], 0.0)

        # chunk list: small first/last chunks prime and drain the
        # pipeline faster; middle chunks are full 4-group superbatches.
        CHUNKS = [(512 * i, 512) for i in range(15)]
        NCH = len(CHUNKS)

        def issue_transposes(ci):
            row0, nr = CHUNKS[ci]
            tiles = []
            for c in range(C):
                yt = work.tile([128, 4, SB_ROWS], f16, tag="yt", bufs=8)
                nc.sync.dma_start(
                    out=yt[:, :, :nr],
                    in_=xf[c, row0 : row0 + nr, :],
                    transpose=True,
                )
                tiles.append(yt)
            return tiles

        def emit_casts(ci, yts):
            """bf16 -> fp8 casts (emitted two chunks early so they never
            queue behind tail-stage work on ACT/DVE/Pool)."""
            if not rfft_fp8:
                return [None, None]
            row0, nr = CHUNKS[ci]
            yt8s = []
            for c in range(C):
                yt = yts[c]
                yt8 = work.tile([128, 4, SB_ROWS], f8, tag="yt8", bufs=8)
                nc.scalar.activation(
                    yt8[:, 0, :nr], yt[:, 0, :nr], AF.Copy, bias=0.0, scale=1.0
                )
                nc.scalar.activation(
                    yt8[:, 1, :nr], yt[:, 1, :nr], AF.Copy, bias=0.0, scale=1.0
                )
                nc.vector.tensor_copy(yt8[:, 2, :nr], yt[:, 2, :nr])
                nc.gpsimd.tensor_copy(yt8[:, 3, :nr], yt[:, 3, :nr])
                yt8s.append(yt8)
            return yt8s

        def emit_front(ci, yts, yt8s):
            """rfft + P for both channels; returns per-c ph state."""
            row0, nr = CHUNKS[ci]
            state = []
            for c in range(C):
                yt = yts[c]
                yt8 = yt8s[c]
                if irfft_fp8:
                    # DR stationary needs the two 128-bin planes interleaved
                    ph8pair = work.tile([128, 2, SB_ROWS], f8, tag="ph8", bufs=4)

                    def ph_ap(h, rows=slice(None), cols=slice(None), _t=ph8pair):
                        return _t[rows, h, cols]
                else:
                    ph8pair = None
                    ph_t0 = work.tile([128, SB_ROWS], phdt, tag="ph0", bufs=4)
                    ph_t1 = work.tile([128, SB_ROWS], phdt, tag="ph1", bufs=4)

                    def ph_ap(
                        h, rows=slice(None), cols=slice(None), _t=(ph_t0, ph_t1)
                    ):
                        return _t[h][rows, cols]

                p256 = None
                for h in range(2):
                    rp = pp.tile([128, SB_ROWS], f32, tag="fft", bufs=5)
                    ip = pp.tile([128, SB_ROWS], f32, tag="fft", bufs=5)
                    if rfft_fp8:
                        for pr in range(2):
                            nc.tensor.matmul(
                                rp[:, :nr],
                                wc_sb[:, pr, :, 128 * h : 128 * h + 128],
                                yt8[:, 2 * pr : 2 * pr + 2, :nr],
                                start=(pr == 0),
                                stop=(pr == 1),
                                perf_mode=DRM,
                            )
                        for pr in range(2):
                            nc.tensor.matmul(
                                ip[:, :nr],
                                ws_sb[:, pr, :, 128 * h : 128 * h + 128],
                                yt8[:, 2 * pr : 2 * pr + 2, :nr],
                                start=(pr == 0),
                                stop=(pr == 1),
                                perf_mode=DRM,
                            )
                    else:
                        for k in range(4):
                            nc.tensor.matmul(
                                rp[:, :nr],
                                wc_sb[:, k, 128 * h : 128 * h + 128],
                                yt[:, k, :nr],
                                start=(k == 0),
                                stop=(k == 3),
                            )
                        for k in range(4):
                            nc.tensor.matmul(
                                ip[:, :nr],
                                ws_sb[:, k, 128 * h : 128 * h + 128],
                                yt[:, k, :nr],
                                start=(k == 0),
                                stop=(k == 3),
                            )
                    sq_r = work.tile([128, SB_ROWS], f16, tag="sqr", bufs=4)
                    nc.scalar.activation(
                        sq_r[:, :nr], rp[:, :nr], AF.Square,
                        bias=zero_b[:], scale=sact,
                    )
                    sq_i = work.tile([128, SB_ROWS], f16, tag="sqi", bufs=8)
                    nc.scalar.activation(
                        sq_i[:, :nr], ip[:, :nr], AF.Square,
                        bias=zero_b[:], scale=sact,
                    )
                    cl = slice(0, nr)
                    nc.vector.tensor_add(
                        ph_ap(h, cols=cl), sq_r[:, :nr], sq_i[:, :nr]
                    )
                    if h == 0:
                        # ph0 row 0 must be Re0^2 (the add wrongly included
                        # the bin-256 cos column carried in ws col 0)
                        nc.vector.tensor_copy(
                            ph_ap(0, rows=slice(0, 1), cols=cl), sq_r[0:1, :nr]
                        )
                        p256 = sq_i  # row 0 = (WS8*X256)^2
                state.append((ph_ap, p256, ph8pair))
            return state

        def emit_tail(ci, state):
            """irfft + norm + channel mean + output DMA for one chunk."""
            row0, nr = CHUNKS[ci]
            ng = nr // 128
            nt0 = []
            mt4 = work.tile([128, 4, LAGS], f16, tag="mt4", bufs=3)
            for c in range(C):
                ph_ap, p256, ph8pair = state[c]
                nts = []
                for j in range(ng):
                    sl = slice(128 * j, 128 * j + 128)
                    acfp = pp.tile([128, LAGS], f32, tag="acf", bufs=3)
                    if irfft_fp8:
                        nc.tensor.matmul(
                            acfp[:],
                            ph8pair[:, :, sl],
                            dm8[:],
                            start=True,
                            stop=False,
                            perf_mode=DRM,
                        )
                    else:
                        nc.tensor.matmul(
                            acfp[:], ph_ap(0, cols=sl), dm0[:],
                            start=True, stop=False,
                        )
                        nc.tensor.matmul(
                            acfp[:], ph_ap(1, cols=sl), dm1[:],
                            start=False, stop=False,
                        )
                    nc.tensor.matmul(
                        acfp[:], p256[0:1, sl], dm2[:],
                        start=False, stop=True,
                    )
                    # rcc = 1/sqrt(4g*acf0 + eps) in one ACT op
                    rcc = work.tile([128, 1], f32, tag="rcc", bufs=8)
                    nc.scalar.activation(
                        rcc[:], acfp[:, 0:1], AF.Abs_reciprocal_sqrt,
                        bias=eps_b[:], scale=s4g,
                    )
                    nt = work.tile(
                        [128, LAGS], f16, tag=f"nt{c}",
                        bufs=(10 if c == 0 else 4),
                    )
                    # nt = max(acfp * rcc, 0) on DVE (one PSUM input)
                    nc.vector.scalar_tensor_tensor(
                        out=nt[:],
                        in0=acfp[:],
                        scalar=rcc[:],
                        in1=zeros_t[:, :LAGS],
                        op0=ALU.mult,
                        op1=ALU.max,
                    )
                    nts.append(nt)
                if c == 0:
                    nt0 = nts
                else:
                    for j in range(ng):
                        nc.vector.tensor_add(mt4[:, j, :], nt0[j][:], nts[j][:])

            nc.sync.dma_start(
                out=out[row0 : row0 + nr, :].rearrange("(j p) l -> p j l", j=ng),
                in_=mt4[:, :ng, :],
            )

        # transposes 3 chunks ahead, casts 2 ahead: per-engine in-order
        # queues never block next-chunk front work behind current tail
        # work, while PE keeps its natural rfft->irfft order.
        yt_queue = {0: issue_transposes(0)}
        load_consts()
        cast_queue = {0: emit_casts(0, yt_queue[0])}
        yt_queue[1] = issue_transposes(1)
        cast_queue[1] = emit_casts(1, yt_queue[1])
        yt_queue[2] = issue_transposes(2)
        for ci in range(NCH):
            if ci + 3 < NCH:
                yt_queue[ci + 3] = issue_transposes(ci + 3)
            if ci + 2 < NCH:
                cast_queue[ci + 2] = emit_casts(ci + 2, yt_queue[ci + 2])
            st = emit_front(ci, yt_queue.pop(ci), cast_queue.pop(ci))
            emit_tail(ci, st)

    nc.compile()
    return nc


_NC_CACHE = {}


def _get_nc(rfft_fp8=False, irfft_fp8=False):
    key = (rfft_fp8, irfft_fp8)
    if key not in _NC_CACHE:
        _NC_CACHE[key] = build_nc(rfft_fp8, irfft_fp8)
    return _NC_CACHE[key]


def make_in_maps(nerv, rfft_fp8=False, irfft_fp8=False):
    import ml_dtypes

    xs = nerv.reshape(B * F, T, C)
    w = build_weights(rfft_fp8, irfft_fp8)
    maps = []
    for i in range(N_CORES):
        xi = xs[BF_PER_CORE * i : BF_PER_CORE * (i + 1)]  # [25, T, 2]
        fr = xi[:, IDX, :]  # [25, 300, 512, 2]
        fr = fr.transpose(3, 1, 0, 2).reshape(C, ROWS, LEN_FRAME)
        xf = np.zeros((C, RPAD, LEN_FRAME), dtype=ml_dtypes.bfloat16)
        xf[:, :ROWS, :] = fr.astype(ml_dtypes.bfloat16)
        maps.append({"xf": xf, **w})
    return maps


def kernel(nervegram, trace=False, rfft_fp8=False, irfft_fp8=False):
    from concourse.bass_utils import run_bass_kernel_spmd

    nerv = np.ascontiguousarray(np.asarray(nervegram, dtype=np.float32))
    assert nerv.shape == (B, F, T, C)
    in_maps = make_in_maps(nerv, rfft_fp8, irfft_fp8)
    nc = _get_nc(rfft_fp8, irfft_fp8)
    res = run_bass_kernel_spmd(nc, in_maps, list(range(N_CORES)), trace=trace)
    outs = []
    for i in range(N_CORES):
        o = np.asarray(res.results[i]["out"])[:ROWS].astype(np.float32)
        outs.append(o.reshape(NUM_FRAME, BF_PER_CORE, LAGS).transpose(1, 0, 2))
    full = np.concatenate(outs, axis=0).reshape(B, F, NUM_FRAME, LAGS)
    if trace:
        return full, res
    return full
